# revision 28
# baseline (speedup 1.0000x reference)
# Self-contained Trainium2 Bass kernel for 16-head MultiHeadAttention
# (B=4, L=2048, HIDDEN=1024, 16 heads x d_k=64), sharded 2 heads per core
# across 8 NeuronCores (tensor-parallel on heads; every core sees all tokens).
#
# Per-core plan (all matmuls bf16 with fp32 PSUM accumulation):
#   x is bf16-cast on the host; x^T tiles built on-device via XBAR transpose DMA
#   Q^T,K^T = W^T-stationary matmuls -> [128 (2 heads x 64), 8192] bf16 (+bias on DVE)
#   V^T -> PE-transpose -> V natural [token-part, 2x(64+ones)] (ones col folds the
#          softmax denominator into the AV matmul)
#   S^T tile = K_tile @ Q^T  (row-tiled pairs: head0 on PE rows 0-63, head1 on
#          rows 64-127 run concurrently via tile_position)
#   P^T = exp(S^T/8) on ScalarE straight from PSUM (no max subtraction: |scores|<~6)
#   att^T[65, Lq] += V_aug^T @ P^T  (V stationary; row 64 = denominator)
#   out = att^T[0:64] * broadcast(1/denominator); stored transposed (2, 64, 8192);
#   the host re-transposes and concatenates heads.

import numpy as np

NUM_HEADS = 16
HIDDEN = 1024
D_K = 64
B = 4
L = 2048
N_CORES = 8
HPC = NUM_HEADS // N_CORES      # heads per core = 2
OPC = HPC * D_K                 # output dims per core = 128

P = 128
T = B * L                       # 8192 tokens
KT = HIDDEN // P                # 8 contraction tiles
TCH = 1024                      # token chunk for x transpose/projection
NCH = T // TCH                  # 8 chunks
LKT = L // P                    # 16 key tiles per batch
QC = 512                        # query chunk (one PSUM bank)
LQC = L // QC                   # 4 query chunks per batch

_CACHE = {}


def _build_nc(reps=1):
    import contextlib

    import concourse.bacc as bacc
    import concourse.mybir as mybir
    import concourse.tile as tile
    from concourse.masks import make_identity

    dt = mybir.dt
    AF = mybir.ActivationFunctionType
    ALU = mybir.AluOpType

    nc = bacc.Bacc(None, target_bir_lowering=False, debug=False)

    # x, bf16-cast on the host (row-major [tokens, hidden])
    x16 = nc.declare_dram_parameter("x16", [T, HIDDEN], dt.bfloat16, isOutput=False)
    wq = nc.declare_dram_parameter("wq", [P, HIDDEN], dt.float32, isOutput=False)
    wk = nc.declare_dram_parameter("wk", [P, HIDDEN], dt.float32, isOutput=False)
    wv = nc.declare_dram_parameter("wv", [P, HIDDEN], dt.float32, isOutput=False)
    bq = nc.declare_dram_parameter("bq", [P, 1], dt.float32, isOutput=False)
    bk = nc.declare_dram_parameter("bk", [P, 1], dt.float32, isOutput=False)
    bv = nc.declare_dram_parameter("bv", [P, 1], dt.float32, isOutput=False)
    out = nc.declare_dram_parameter("out", [HPC, D_K, T], dt.float32, isOutput=True)

    with tile.TileContext(nc) as tc:
        with (
            tc.tile_pool(name="const", bufs=1) as const,
            tc.tile_pool(name="persist", bufs=1) as persist,
            tc.tile_pool(name="wstage", bufs=2) as wstage,
            tc.tile_pool(name="xtp", bufs=2) as xtp,
            tc.tile_pool(name="vtp", bufs=2) as vtp,
            tc.tile_pool(name="ptp", bufs=3) as ptp,
            tc.tile_pool(name="fin", bufs=2) as fin,
            # PSUM budget (8 banks): proj+scores share "mm" 2x2, transposes 2x1,
            # attended accumulators 2x1.
            tc.tile_pool(name="mm", bufs=2, space="PSUM") as mmp,
            tc.tile_pool(name="tp", bufs=2, space="PSUM") as tpp,
            tc.tile_pool(name="avp", bufs=2, space="PSUM") as avp,
        ):
            ident = const.tile([P, P], dt.bfloat16, tag="ident")
            make_identity(nc, ident)

            # --- weights: load, cast to bf16, transpose to [D-part, kt, 128] ---
            wts = []
            bts = []
            for nm, wparam, bparam in (("q", wq, bq), ("k", wk, bk), ("v", wv, bv)):
                wst = wstage.tile([P, HIDDEN], dt.float32, tag="wst")
                nc.sync.dma_start(out=wst[:], in_=wparam[:])
                wbf = wstage.tile([P, HIDDEN], dt.bfloat16, tag="wbf")
                nc.vector.tensor_copy(out=wbf[:], in_=wst[:])
                wt = const.tile([P, KT, P], dt.bfloat16, tag=f"wt{nm}")
                for j in range(KT):
                    ps = tpp.tile([P, P], dt.bfloat16, tag="tp")
                    nc.tensor.transpose(ps[:], wbf[:, j * P:(j + 1) * P], ident[:])
                    nc.vector.tensor_copy(out=wt[:, j, :], in_=ps[:])
                bt = const.tile([P, 1], dt.float32, tag=f"b{nm}")
                nc.sync.dma_start(out=bt[:], in_=bparam[:])
                wts.append(wt)
                bts.append(bt)

            # --- persistent activations ---
            qT = persist.tile([P, T], dt.bfloat16, tag="qT")
            kT = persist.tile([P, T], dt.bfloat16, tag="kT")
            # V natural layout + ones columns: [tok-part, tok-tile, 2*(64+1)]
            vaug = persist.tile([P, T // P, 2 * (D_K + 1)], dt.bfloat16, tag="vaug")
            nc.vector.memset(vaug[:, :, D_K:D_K + 1], 1.0)
            nc.vector.memset(vaug[:, :, 2 * D_K + 1:2 * D_K + 2], 1.0)

            # For timing runs (reps>1) the whole per-call body loops on-device.
            rep_ctx = tc.For_i(0, reps, 1) if reps > 1 else contextlib.nullcontext()
            with rep_ctx:
                # --- x^T chunks via XBAR transpose DMA, then projections ---
                for ch in range(NCH):
                    t0 = ch * TCH
                    xt = xtp.tile([P, KT, TCH], dt.bfloat16, tag="xt")
                    for k in range(KT):
                        nc.sync.dma_start_transpose(
                            xt[:, k, :], x16[t0:t0 + TCH, k * P:(k + 1) * P]
                        )
                    for idx in range(3):
                        ps = mmp.tile([P, TCH], dt.float32, tag="mm")
                        for h2 in range(TCH // QC):
                            for k in range(KT):
                                nc.tensor.matmul(
                                    ps[:, h2 * QC:(h2 + 1) * QC],
                                    lhsT=wts[idx][:, k, :],
                                    rhs=xt[:, k, h2 * QC:(h2 + 1) * QC],
                                    start=(k == 0),
                                    stop=(k == KT - 1),
                                )
                        if idx < 2:
                            dest = qT if idx == 0 else kT
                            nc.vector.tensor_scalar_add(
                                out=dest[:, t0:t0 + TCH], in0=ps[:], scalar1=bts[idx][:]
                            )
                        else:
                            vt = vtp.tile([P, TCH], dt.bfloat16, tag="vt")
                            nc.vector.tensor_scalar_add(
                                out=vt[:], in0=ps[:], scalar1=bts[idx][:]
                            )
                            for j in range(TCH // P):
                                ps2 = tpp.tile([P, P], dt.bfloat16, tag="tp")
                                nc.tensor.transpose(
                                    ps2[:], vt[:, j * P:(j + 1) * P], ident[:]
                                )
                                tt = ch * (TCH // P) + j
                                nc.vector.tensor_copy(
                                    out=vaug[:, tt, 0:D_K], in_=ps2[:, 0:D_K]
                                )
                                nc.vector.tensor_copy(
                                    out=vaug[:, tt, D_K + 1:2 * D_K + 1],
                                    in_=ps2[:, D_K:2 * D_K],
                                )

                # --- attention ---
                for b in range(B):
                    for cq in range(LQC):
                        qs = b * L + cq * QC
                        av0 = avp.tile([P, QC], dt.float32, tag="av")
                        av1 = avp.tile([P, QC], dt.float32, tag="av")
                        for lk in range(LKT):
                            ks = b * L + lk * P
                            st = mmp.tile([P, 2, QC], dt.float32, tag="mm")
                            nc.tensor.matmul(
                                st[:, 0, :], lhsT=kT[0:D_K, ks:ks + P],
                                rhs=qT[0:D_K, qs:qs + QC],
                                start=True, stop=True, tile_position=(0, 0),
                            )
                            nc.tensor.matmul(
                                st[:, 1, :], lhsT=kT[D_K:P, ks:ks + P],
                                rhs=qT[D_K:P, qs:qs + QC],
                                start=True, stop=True, tile_position=(64, 0),
                            )
                            pt = ptp.tile([P, 2, QC], dt.bfloat16, tag="pt")
                            nc.scalar.activation(
                                out=pt[:], in_=st[:], func=AF.Exp,
                                scale=1.0 / np.sqrt(D_K),
                            )
                            ltile = b * LKT + lk
                            nc.tensor.matmul(
                                av0[:D_K + 1, :], lhsT=vaug[:, ltile, 0:D_K + 1],
                                rhs=pt[:, 0, :],
                                start=(lk == 0), stop=(lk == LKT - 1),
                            )
                            nc.tensor.matmul(
                                av1[:D_K + 1, :],
                                lhsT=vaug[:, ltile, D_K + 1:2 * (D_K + 1)],
                                rhs=pt[:, 1, :],
                                start=(lk == 0), stop=(lk == LKT - 1),
                            )
                        for h, av in ((0, av0), (1, av1)):
                            rc = fin.tile([1, QC], dt.float32, tag="rc")
                            nc.vector.reciprocal(rc[:], av[D_K:D_K + 1, :])
                            bc = fin.tile([D_K, QC], dt.float32, tag="bc")
                            nc.gpsimd.partition_broadcast(bc[:], rc[:])
                            osb = fin.tile([D_K, QC], dt.float32, tag="osb")
                            nc.vector.tensor_tensor(
                                osb[:], av[0:D_K, :], bc[:], ALU.mult
                            )
                            nc.sync.dma_start(out=out[h, :, qs:qs + QC], in_=osb[:])

    nc.compile()
    return nc


def get_nc(reps=1, **kw):
    key = f"nc{reps}-{sorted(kw.items())}"
    if key not in _CACHE:
        _CACHE[key] = _build_nc(reps, **kw)
    return _CACHE[key]


def _shard_inputs(x, Wq, bq, Wk, bk, Wv, bv):
    import ml_dtypes

    x2d = np.ascontiguousarray(
        np.asarray(x, dtype=np.float32).reshape(T, HIDDEN).astype(ml_dtypes.bfloat16)
    )
    in_maps = []
    for c in range(N_CORES):
        sl = slice(c * OPC, (c + 1) * OPC)
        in_maps.append({
            "x16": x2d,
            "wq": np.ascontiguousarray(np.asarray(Wq, dtype=np.float32)[sl]),
            "wk": np.ascontiguousarray(np.asarray(Wk, dtype=np.float32)[sl]),
            "wv": np.ascontiguousarray(np.asarray(Wv, dtype=np.float32)[sl]),
            "bq": np.ascontiguousarray(np.asarray(bq, dtype=np.float32)[sl].reshape(P, 1)),
            "bk": np.ascontiguousarray(np.asarray(bk, dtype=np.float32)[sl].reshape(P, 1)),
            "bv": np.ascontiguousarray(np.asarray(bv, dtype=np.float32)[sl].reshape(P, 1)),
        })
    return in_maps


def _gather(results):
    att = np.empty((B, NUM_HEADS, L, D_K), dtype=np.float32)
    for c in range(N_CORES):
        r = results[c]["out"]  # (HPC, D_K, T)
        for h in range(HPC):
            att[:, c * HPC + h] = r[h].T.reshape(B, L, D_K)
    return att


def run(x, Wq, bq, Wk, bk, Wv, bv, trace=False):
    from concourse.bass_utils import run_bass_kernel_spmd

    nc = get_nc()
    in_maps = _shard_inputs(x, Wq, bq, Wk, bk, Wv, bv)
    res = run_bass_kernel_spmd(
        nc, in_maps, core_ids=list(range(N_CORES)), trace=trace
    )
    return _gather(res.results), res


def kernel(x, Wq, bq, Wk, bk, Wv, bv):
    att, _ = run(x, Wq, bq, Wk, bk, Wv, bv, trace=False)
    return att
